# revision 41
# baseline (speedup 1.0000x reference)
"""Root-to-leaves TreeLSTM over a complete binary tree (depth 17, 131071 nodes,
feat=h=512), distributed over 8 TRN2 NeuronCores with zero inter-core
communication.

Sharding: level d's nodes split into 8 contiguous chunks means each core's
chunk at level d+1 is exactly the children of its chunk at level d, so each
core owns one of the 8 subtrees rooted at level 3. Levels 0-3 are replicated
on all cores; the SPMD program relabels them per-core by XOR with the core
index prefix so "my subtree root" is column 0 everywhere and the parent map
is position-independent (parent of col j is col j//2 in plain node order).

v2: fp8 DoubleRow matmuls (2x PE throughput) for the iofu x-GEMM and h-GEMM,
accumulated into a 2-bank PSUM pair per (gate, k-tile-of-h). The px GEMM
stays bf16 for accuracy (its error passes to the output unattenuated by any
sigmoid). Weights are pre-scaled into fp8's normal range (wx*64, wh*64) and
compensated by the activation scale (1/64). Gates, c state, tmps and the
output are bf16 (2x DVE, half the output DMA); h state is fp8.

Two per-level layouts:
- small levels (<= 256 cols): node order (parent of col j is col j//2), all
  4 k-tiles packed into one psum pair per gate (one activation each), and
  the h-GEMM reads parent h through a stride-0 repeat AP so each parent
  feeds both children directly.
- big levels: block order ([L-children | R-children], parent of col j is
  col j mod npar) so the h-GEMM runs ONCE per parent into the L psum bank;
  a DVE copy evacuates the 64-scaled gh to SBUF and an identity matmul
  accumulates it into the R bank (halves the h-GEMM tensor work). Gates are
  emitted in (i, u, f, o, r) order with the c/tanh chain interleaved so only
  e/hf/h-store trail the last activation, and injects lag their gh copy by
  two gates to hide DVE latency.
"""

import os
import sys

sys.path.insert(0, "/opt/trn_rl_repo")

import numpy as np
import ml_dtypes
from contextlib import ExitStack

import concourse.bass as bass
import concourse.mybir as mybir
import concourse.tile as tile
from concourse import bacc

P = 128
KT = 4               # 512 / 128 contraction tiles
H = 512
F = 512
DEPTH = 17
NCORES = 8
CHUNK = 1024         # children columns per chunk
PCH = 512            # parents per block-order chunk (=> CHUNK children)
M_IOFU = 20          # iofu M-tiles (2560/128), fp8
M_PX = 4             # px M-tiles (512/128), bf16
WSCALE = 64.0        # wx fp8 pre-scale
WHSCALE = 64.0       # wh fp8 pre-scale (h state stored unscaled)
TPACK_W = 256        # pack all 4 k-tiles into one psum pair when w <= this
BF16 = mybir.dt.bfloat16
FP8 = mybir.dt.float8e4
F32 = mybir.dt.float32
AF = mybir.ActivationFunctionType
DR = mybir.MatmulPerfMode.DoubleRow
np_bf16 = ml_dtypes.bfloat16
np_fp8 = ml_dtypes.float8_e4m3


def _level_sizes(depth):
    # per-core column count per level: levels 0..3 replicated, >=4 core-private
    return [1 << d if d <= 3 else 1 << (d - 3) for d in range(depth)]


def _plan(depth):
    """Level sizes, feature-column offsets, and per-parity state-buffer
    widths (level d stores into buffer d % 2; no level splitting — the two
    buffers are sized for the largest even/odd stored level)."""
    Ns = _level_sizes(depth)
    off = [0]
    for n in Ns:
        off.append(off[-1] + n)
    sc = [1, 1]
    for d in range(depth - 1):
        sc[d % 2] = max(sc[d % 2], Ns[d])
    return Ns, off, sc


def build_nc(depth=DEPTH):
    """Build the SPMD single-core Bass program (same NEFF for all 8 cores)."""
    Ns, off, sc = _plan(depth)
    C = off[-1]

    nc = bacc.Bacc("TRN2", target_bir_lowering=False, debug=False)
    featsq = nc.declare_dram_parameter("featsq", [F, C], FP8, isOutput=False)
    featsb = nc.declare_dram_parameter("featsb", [F, C], BF16, isOutput=False)
    wxq = nc.declare_dram_parameter("wxq", [F, M_IOFU * P], FP8, isOutput=False)
    wxpx = nc.declare_dram_parameter("wxpx", [F, M_PX * P], BF16, isOutput=False)
    whq = nc.declare_dram_parameter("whq", [H, M_IOFU * P], FP8, isOutput=False)
    biasm = nc.declare_dram_parameter("biasm", [P, M_IOFU], F32, isOutput=False)
    pxb = nc.declare_dram_parameter("pxb", [P, M_PX], F32, isOutput=False)
    ident = nc.declare_dram_parameter("ident", [P, P], BF16, isOutput=False)
    outT = nc.declare_dram_parameter("outT", [H, C], BF16, isOutput=True)

    featsq_r = featsq[:].rearrange("(a p) c -> p a c", p=P)
    featsb_r = featsb[:].rearrange("(a p) c -> p a c", p=P)
    wxq_r = wxq[:].rearrange("(a p) m -> p a m", p=P)
    wxpx_r = wxpx[:].rearrange("(a p) m -> p a m", p=P)
    whq_r = whq[:].rearrange("(a p) m -> p a m", p=P)
    outT_r = outT[:].rearrange("(a p) c -> p a c", p=P)

    with ExitStack() as ctx:
        tc = ctx.enter_context(tile.TileContext(nc))
        wpool = ctx.enter_context(tc.tile_pool(name="w", bufs=1))
        spool = ctx.enter_context(tc.tile_pool(name="state", bufs=1))
        fpool = ctx.enter_context(tc.tile_pool(name="feats", bufs=4))
        pspool = ctx.enter_context(tc.tile_pool(name="ps", bufs=4, space="PSUM"))
        gpool = ctx.enter_context(tc.tile_pool(name="gates", bufs=10))
        xpool = ctx.enter_context(tc.tile_pool(name="px", bufs=4))
        tpool = ctx.enter_context(tc.tile_pool(name="tmp", bufs=8))
        opool = ctx.enter_context(tc.tile_pool(name="hf", bufs=5))
        ghpool = ctx.enter_context(tc.tile_pool(name="gh", bufs=5))

        wx_sb = wpool.tile([P, KT, M_IOFU * P], FP8, tag="wxq")
        wpx_sb = wpool.tile([P, KT, M_PX * P], BF16, tag="wxpx")
        wh_sb = wpool.tile([P, KT, M_IOFU * P], FP8, tag="whq")
        bias_sb = wpool.tile([P, M_IOFU], F32, tag="biasm")
        pxb_sb = wpool.tile([P, M_PX], F32, tag="pxb")
        id_sb = wpool.tile([P, P], BF16, tag="ident")
        # small tensors first, big weights split by k-tile pair so the first
        # matmuls (kp=0) can start after half the weight traffic
        nc.sync.dma_start(bias_sb[:], biasm[:])
        nc.sync.dma_start(pxb_sb[:], pxb[:])
        nc.sync.dma_start(id_sb[:], ident[:])
        nc.sync.dma_start(wx_sb[:, 0:2, :], wxq_r[:, 0:2, :])
        nc.sync.dma_start(wpx_sb[:, 0:2, :], wxpx_r[:, 0:2, :])
        nc.sync.dma_start(wh_sb[:, 0:2, :], whq_r[:, 0:2, :])
        nc.sync.dma_start(wx_sb[:, 2:4, :], wxq_r[:, 2:4, :])
        nc.sync.dma_start(wpx_sb[:, 2:4, :], wxpx_r[:, 2:4, :])
        nc.sync.dma_start(wh_sb[:, 2:4, :], whq_r[:, 2:4, :])

        # state double buffers (level d -> buffer d % 2): c bf16, h fp8
        cst = [spool.tile([P, KT, sc[b]], BF16, tag=f"c{b}", name=f"c{b}")
               for b in (0, 1)]
        hst = [spool.tile([P, KT, sc[b]], FP8, tag=f"h{b}", name=f"h{b}")
               for b in (0, 1)]

        def elemwise_early(d, w, gi_, gu_, gf_, px, pc_reps, c_dst, tv):
            """First part of the bf16 elementwise chain (after i/u/f acts):
            c = i*u + f*pc and tanh(c). Runs while o/r GEMMs proceed."""
            if d > 0:
                t1 = tpool.tile([P, CHUNK], BF16, tag="tmp")
                t2 = tpool.tile([P, CHUNK], BF16, tag="tmp")
                nc.vector.tensor_mul(t1[:, :w], gi_[:, :w], gu_[:, :w])
                for sl, vfn, pc_rep in pc_reps:
                    nc.gpsimd.tensor_mul(vfn(t2[:, sl]), vfn(gf_[:, sl]),
                                         pc_rep)
                nc.vector.tensor_add(c_dst, tv(t1[:, :w]), tv(t2[:, :w]))
            else:
                nc.vector.tensor_mul(c_dst, tv(gi_[:, :w]), tv(gu_[:, :w]))
            tc_ = tpool.tile([P, CHUNK], BF16, tag="tmp")
            nc.scalar.activation(tv(tc_[:, :w]), c_dst, AF.Tanh)
            return tc_

        def elemwise_mid(w, go_, tc_, px):
            """After o's act: t3 = o*tanh(c); d = t3 - px."""
            t3 = tpool.tile([P, CHUNK], BF16, tag="tmp")
            nc.vector.tensor_mul(t3[:, :w], go_[:, :w], tc_[:, :w])
            d_ = tpool.tile([P, CHUNK], BF16, tag="tmp")
            nc.vector.tensor_sub(d_[:, :w], t3[:, :w], px[:, :w])
            return d_

        def elemwise_late(w, gr_, d_, px, tv, h_dst, out_ap):
            """After r's act (critical tail): e = r*d; hf = e + px; store."""
            e_ = tpool.tile([P, CHUNK], BF16, tag="tmp")
            nc.vector.tensor_mul(e_[:, :w], gr_[:, :w], d_[:, :w])
            hf = opool.tile([P, CHUNK], BF16, tag="hf")
            nc.vector.tensor_add(hf[:, :w], e_[:, :w], px[:, :w])
            if h_dst is not None:
                nc.vector.tensor_copy(h_dst, tv(hf[:, :w]))
            nc.sync.dma_start(out_ap, tv(hf[:, :w]))

        def chunk_tpack(d, col0, st0, pp0, w, store, buf, ftq, ftb):
            """Small-level chunk (w <= TPACK_W): all 4 k-tiles packed into
            one psum pair per gate -> one activation per gate, 3-dim state
            APs, 4x fewer elementwise/activation ops."""
            pbuf = (d - 1) % 2
            hw = w // 2
            tw = KT * w
            tpb = max(1, 512 // w)  # t-slices per 2KB psum bank

            def first_t(t):
                return t % tpb == 0

            def last_t(t):
                return t % tpb == tpb - 1 or t == KT - 1

            ps_px = pspool.tile([P, CHUNK], F32, tag="ps")
            for t in range(KT):
                for k in range(KT):
                    nc.tensor.matmul(
                        ps_px[:, t * w:(t + 1) * w],
                        wpx_sb[:, k, t * P:(t + 1) * P], ftb[:, k, :w],
                        start=(first_t(t) and k == 0),
                        stop=(last_t(t) and k == KT - 1))
            px = xpool.tile([P, CHUNK], BF16, tag="px")
            nc.scalar.activation(px[:, :tw], ps_px[:, :tw], AF.Identity,
                                 bias=pxb_sb[:, 0:1])

            def gate_x(gi, ps):
                """x-parts only (feature-dependent): opens each psum bank."""
                for t in range(KT):
                    m = gi * KT + t
                    for kp in (0, 2):
                        nc.tensor.matmul(
                            ps[:, t * w:(t + 1) * w],
                            wx_sb[:, kp:kp + 2, m * P:(m + 1) * P],
                            ftq[:, kp:kp + 2, :w], perf_mode=DR,
                            start=(first_t(t) and kp == 0),
                            stop=(last_t(t) and kp == 2 and d == 0))

            def gate_h(gi, ps):
                """h-parts (parent-state-dependent): closes the banks."""
                for t in range(KT):
                    m = gi * KT + t
                    for kp in (0, 2):
                        hrep = hst[pbuf][
                            :, kp:kp + 2, pp0:pp0 + hw,
                            None].to_broadcast((P, 2, hw, 2))
                        nc.tensor.matmul(
                            ps[:, t * w:t * w + 2 * hw],
                            wh_sb[:, kp:kp + 2, m * P:(m + 1) * P],
                            hrep, perf_mode=DR,
                            start=False, stop=(last_t(t) and kp == 2))

            def gate_act(gi, ps):
                g = gpool.tile([P, CHUNK], BF16, tag="gates")
                func = AF.Tanh if gi == 3 else AF.Sigmoid
                nc.scalar.activation(g[:, :tw], ps[:, :tw], func,
                                     bias=bias_sb[:, gi * KT:gi * KT + 1],
                                     scale=1.0 / WSCALE)
                return g

            if store:
                c_dst = cst[buf][:, :, st0:st0 + w]
                h_dst = hst[buf][:, :, st0:st0 + w]
            else:
                c_dst = tpool.tile([P, CHUNK], BF16, tag="tmp",
                                   name="ctmp")[:, :tw].rearrange(
                                       "p (a b) -> p a b", a=KT)
                h_dst = None
            tv = lambda ap: ap.rearrange("p (a b) -> p a b", a=KT)
            nodev = lambda ap: ap.rearrange("p (a b) -> p a b", b=2)
            pc_reps = []
            if d > 0:
                for t in range(KT):
                    pc_reps.append((
                        slice(t * w, (t + 1) * w), nodev,
                        cst[pbuf][:, t, pp0:pp0 + hw, None].to_broadcast(
                            (P, hw, 2))))
            # software pipeline: x-parts run ahead (feature-only deps) so the
            # PE has queued work while the parent level's h-store completes
            acted = {}
            pss = {}
            pipe = []

            def retire(g0):
                if d > 0:
                    gate_h(g0, pss[g0])
                acted[g0] = gate_act(g0, pss[g0])

            for gi in [0, 3, 2, 1, 4]:
                if len(pipe) >= 3:
                    retire(pipe.pop(0))
                ps = pspool.tile([P, CHUNK], F32, tag="ps")
                pss[gi] = ps
                gate_x(gi, ps)
                pipe.append(gi)
            retire(pipe.pop(0))  # f
            tc_ = elemwise_early(d, tw, acted[0], acted[3], acted[2], px,
                                 pc_reps, c_dst, tv)
            retire(pipe.pop(0))  # o
            d_ = elemwise_mid(tw, acted[1], tc_, px)
            retire(pipe.pop(0))  # r
            elemwise_late(tw, acted[4], d_, px, tv, h_dst,
                          outT_r[:, :, col0:col0 + w])

        def chunk_blk(d, coloff, npar, p0, pw, store, buf):
            """Block-order chunk for big levels (level size > TPACK_W):
            process parents [p0, p0+pw); children laid out [L-block |
            R-block] within the level. Per gate: h-GEMM once into the L
            half of a 2-bank psum pair, R x-GEMM into the R half as its
            own group, one DVE add seeds gh into R, then the L x-GEMM
            accumulates on top of gh. Halves the h-GEMM tensor work."""
            pbuf = (d - 1) % 2
            w2 = 2 * pw
            ftq = fpool.tile([P, KT, CHUNK], FP8, tag="ftq")
            ftb = fpool.tile([P, KT, CHUNK], BF16, tag="ftb")
            cL = coloff + p0
            cR = coloff + npar + p0
            nc.sync.dma_start(ftq[:, :, 0:pw], featsq_r[:, :, cL:cL + pw])
            nc.sync.dma_start(ftq[:, :, pw:w2], featsq_r[:, :, cR:cR + pw])
            nc.sync.dma_start(ftb[:, :, 0:pw], featsb_r[:, :, cL:cL + pw])
            nc.sync.dma_start(ftb[:, :, pw:w2], featsb_r[:, :, cR:cR + pw])
            for t in range(KT):
                ps_px = pspool.tile([P, CHUNK], F32, tag="ps")
                for k in range(KT):
                    for c0 in range(0, w2, 512):
                        cw = min(512, w2 - c0)
                        nc.tensor.matmul(
                            ps_px[:, c0:c0 + cw],
                            wpx_sb[:, k, t * P:(t + 1) * P],
                            ftb[:, k, c0:c0 + cw],
                            start=(k == 0 and c0 % 512 == 0),
                            stop=(k == KT - 1
                                  and (c0 + cw == w2 or (c0 + cw) % 512 == 0)))
                px = xpool.tile([P, CHUNK], BF16, tag="px")
                nc.scalar.activation(px[:, :w2], ps_px[:, :w2], AF.Identity,
                                     bias=pxb_sb[:, t:t + 1])

                def gate_h_rx(gi, ps):
                    """R-children x-GEMM into [pw:w2] and h-GEMM into [0:pw]
                    (group opened, not closed). When the two halves live in
                    separate psum banks (pw == 512), the Rx part is emitted
                    first: it only needs features, so the in-order PE queue
                    can chew on it while the parent level's h-state store
                    completes."""
                    m = gi * KT + t
                    rx_first = pw % 512 == 0

                    def emit_h():
                        for kp in (0, 2):
                            for q0 in range(0, pw, 256):
                                qw = min(256, pw - q0)
                                nc.tensor.matmul(
                                    ps[:, q0:q0 + qw],
                                    wh_sb[:, kp:kp + 2, m * P:(m + 1) * P],
                                    hst[pbuf][:, kp:kp + 2,
                                              p0 + q0:p0 + q0 + qw],
                                    perf_mode=DR,
                                    start=(kp == 0 and q0 % 512 == 0),
                                    stop=False)

                    def emit_rx():
                        for kp in (0, 2):
                            for c0 in range(pw, w2, 256):
                                cw = min(256, w2 - c0)
                                nc.tensor.matmul(
                                    ps[:, c0:c0 + cw],
                                    wx_sb[:, kp:kp + 2, m * P:(m + 1) * P],
                                    ftq[:, kp:kp + 2, c0:c0 + cw],
                                    perf_mode=DR,
                                    start=(kp == 0 and c0 % 512 == 0),
                                    stop=False)

                    if rx_first:
                        emit_rx()
                        emit_h()
                    else:
                        emit_h()
                        emit_rx()

                def gate_lx(gi, ps):
                    """L-children x-GEMM accumulating onto gh in [0:pw];
                    closes the bank(s) the h-GEMM opened."""
                    m = gi * KT + t
                    for kp in (0, 2):
                        for c0 in range(0, pw, 256):
                            cw = min(256, pw - c0)
                            nc.tensor.matmul(
                                ps[:, c0:c0 + cw],
                                wx_sb[:, kp:kp + 2, m * P:(m + 1) * P],
                                ftq[:, kp:kp + 2, c0:c0 + cw],
                                perf_mode=DR, start=False,
                                stop=(kp == 2 and (c0 + cw == pw
                                                   or (c0 + cw) % 512 == 0)))

                def gate_act(gi, ps):
                    g = gpool.tile([P, CHUNK], BF16, tag="gates")
                    func = AF.Tanh if gi == 3 else AF.Sigmoid
                    nc.scalar.activation(g[:, :w2], ps[:, :w2], func,
                                         bias=bias_sb[:, gi * KT:gi * KT + 1],
                                         scale=1.0 / WSCALE)
                    return g

                if store:
                    c_dst = cst[buf][:, t, 0:2 * npar].rearrange(
                        "p (b q) -> p b q", b=2)[:, :, p0:p0 + pw]
                    h_dst = hst[buf][:, t, 0:2 * npar].rearrange(
                        "p (b q) -> p b q", b=2)[:, :, p0:p0 + pw]
                else:
                    c_dst = tpool.tile([P, CHUNK], BF16, tag="tmp",
                                       name="ctmp")[:, :w2].rearrange(
                                           "p (b q) -> p b q", b=2)
                    h_dst = None
                tv = lambda ap: ap.rearrange("p (b q) -> p b q", b=2)
                out_ap = outT_r[:, t, coloff:coloff + 2 * npar].rearrange(
                    "p (b q) -> p b q", b=2)[:, :, p0:p0 + pw]
                blkv = lambda ap: ap.rearrange("p (b q) -> p b q", b=2)
                pc_reps = [(slice(0, w2), blkv,
                            cst[pbuf][:, t, None, p0:p0 + pw].to_broadcast(
                                (P, 2, pw)))]

                def inject(gi):
                    # accumulate the 64-scaled gh into the R half via an
                    # identity matmul from the SBUF copy
                    nc.tensor.matmul(pss[gi][:, pw:w2], id_sb[:], ghs[gi],
                                     start=False, stop=(pw % 512 == 0))

                # software-pipelined gate sequence (i, u, f, o, r) with
                # 2-gate lag between the gh copy and its inject
                acted = {}
                order = [0, 3, 2, 1, 4]
                pss = {}
                ghs = {}
                pipe = []

                def retire(g0):
                    inject(g0)
                    gate_lx(g0, pss[g0])
                    acted[g0] = gate_act(g0, pss[g0])

                for gi in order:
                    if len(pipe) >= 3:
                        retire(pipe.pop(0))
                    ps = pspool.tile([P, CHUNK], F32, tag="ps")
                    pss[gi] = ps
                    gate_h_rx(gi, ps)
                    gh = ghpool.tile([P, PCH], BF16, tag="gh",
                                     name="gh")[:, :pw]
                    nc.vector.tensor_copy(gh, ps[:, 0:pw])
                    ghs[gi] = gh
                    pipe.append(gi)
                retire(pipe.pop(0))  # f
                tc_ = elemwise_early(d, w2, acted[0], acted[3], acted[2],
                                     px, pc_reps, c_dst, tv)
                retire(pipe.pop(0))  # o
                d_ = elemwise_mid(w2, acted[1], tc_, px)
                retire(pipe.pop(0))  # r
                elemwise_late(w2, acted[4], d_, px, tv, h_dst, out_ap)

        for d in range(depth):
            store = d < depth - 1
            buf = d % 2
            n = Ns[d]
            if n <= TPACK_W:
                ftq = fpool.tile([P, KT, CHUNK], FP8, tag="ftq")
                ftb = fpool.tile([P, KT, CHUNK], BF16, tag="ftb")
                nc.sync.dma_start(ftq[:, :, :n],
                                  featsq_r[:, :, off[d]:off[d] + n])
                nc.sync.dma_start(ftb[:, :, :n],
                                  featsb_r[:, :, off[d]:off[d] + n])
                chunk_tpack(d, off[d], 0, 0, n, store, buf, ftq, ftb)
            else:
                npar = n // 2
                for p0 in range(0, npar, PCH):
                    pw = min(PCH, npar - p0)
                    chunk_blk(d, off[d], npar, p0, pw, store, buf)

    nc.compile()
    return nc, C


# ---------------------------------------------------------------- host side

def _col_maps(depth):
    """Per (core, level): global node index for each column. Small levels
    (size <= TPACK_W) use node order (children interleaved: parent of col j
    is col j//2); big levels use block order ([L-children | R-children]:
    parent of col j is col j mod npar). Levels 0-3 are replicated with a
    per-core child-swap so each core's subtree root lands at column 0 of
    level 3."""
    Ns = _level_sizes(depth)
    maps = []
    for i in range(NCORES):
        per_level = []
        cur = np.array([0], dtype=np.int64)
        for d in range(depth):
            if d == 0:
                cur = np.array([0], dtype=np.int64)
            elif d == 4:
                # first private level: children of this core's subtree root
                r = cur[0]
                cur = np.array([2 * r, 2 * r + 1], dtype=np.int64)
            else:
                L, R = 2 * cur, 2 * cur + 1
                if d <= 3:
                    if (i >> (3 - d)) & 1:
                        L, R = R, L
                    cur = np.stack([L, R], axis=1).ravel()
                elif Ns[d] <= TPACK_W:
                    cur = np.stack([L, R], axis=1).ravel()
                else:
                    cur = np.concatenate([L, R])
            per_level.append(((1 << d) - 1) + cur)
        maps.append(per_level)
    return maps


def prep_inputs(features, px_w, px_b, iofux_w, iofux_b, iofuh_w, iofuh_b,
                depth=DEPTH):
    Ns = _level_sizes(depth)
    C = sum(Ns)
    features = np.asarray(features, np.float32)
    wxq = np.ascontiguousarray(
        (np.asarray(iofux_w, np.float32) * WSCALE).T).astype(np_fp8)
    wxpx = np.ascontiguousarray(
        np.asarray(px_w, np.float32).T).astype(np_bf16)
    whq = np.ascontiguousarray(
        (np.asarray(iofuh_w, np.float32) * WHSCALE).T).astype(np_fp8)
    bias_all = np.asarray(iofux_b, np.float32) + np.asarray(iofuh_b, np.float32)
    biasm = np.ascontiguousarray(bias_all.reshape(M_IOFU, P).T)
    pxbm = np.ascontiguousarray(
        np.asarray(px_b, np.float32).reshape(M_PX, P).T)

    maps = _col_maps(depth)
    idm = np.eye(P, dtype=np_bf16)
    in_maps = []
    for i in range(NCORES):
        cols = np.concatenate(maps[i])
        fcore = features[cols, :]                       # [C, 512] f32
        fT = np.ascontiguousarray(fcore.T)              # [512, C]
        in_maps.append({"featsq": fT.astype(np_fp8),
                        "featsb": fT.astype(np_bf16),
                        "wxq": wxq, "wxpx": wxpx, "whq": whq,
                        "biasm": biasm, "pxb": pxbm, "ident": idm})
    return in_maps, maps, C


def assemble_output(results, maps, depth=DEPTH):
    Ns = _level_sizes(depth)
    n_nodes = (1 << depth) - 1
    out = np.empty((n_nodes, H), np.float32)
    offs = np.cumsum([0] + Ns)
    for i in range(NCORES):
        o = results[i]["outT"]                          # [512, C] bf16
        for d in range(depth):
            if d <= 3 and i != 0:
                continue  # replicated levels: take core 0's copy
            cols = maps[i][d]
            out[cols, :] = o[:, offs[d]:offs[d + 1]].T.astype(np.float32)
    return out


_CACHE = {}


def _get_built(depth=DEPTH):
    if depth not in _CACHE:
        _CACHE[depth] = build_nc(depth)
    return _CACHE[depth]


def run_cores(in_maps, depth=DEPTH, trace=False):
    from concourse.bass_utils import run_bass_kernel_spmd
    nc, C = _get_built(depth)
    br = run_bass_kernel_spmd(nc, in_maps, list(range(NCORES)), trace=trace)
    return br


def kernel(features, px_w, px_b, iofux_w, iofux_b, iofuh_w, iofuh_b):
    in_maps, maps, C = prep_inputs(features, px_w, px_b, iofux_w, iofux_b,
                                   iofuh_w, iofuh_b)
    br = run_cores(in_maps)
    return assemble_output(br.results, maps)


# revision 42
# speedup vs baseline: 1.1959x; 1.1959x over previous
"""Root-to-leaves TreeLSTM over a complete binary tree (depth 17, 131071 nodes,
feat=h=512), distributed over 8 TRN2 NeuronCores with zero inter-core
communication.

Sharding: level d's nodes split into 8 contiguous chunks means each core's
chunk at level d+1 is exactly the children of its chunk at level d, so each
core owns one of the 8 subtrees rooted at level 3. Levels 0-3 are replicated
on all cores; the SPMD program relabels them per-core by XOR with the core
index prefix so "my subtree root" is column 0 everywhere and the parent map
is position-independent (parent of col j is col j//2 in plain node order).

v2: fp8 DoubleRow matmuls (2x PE throughput) for the iofu x-GEMM and h-GEMM,
accumulated into a 2-bank PSUM pair per (gate, k-tile-of-h). The px GEMM
stays bf16 for accuracy (its error passes to the output unattenuated by any
sigmoid). Weights are pre-scaled into fp8's normal range (wx*64, wh*64) and
compensated by the activation scale (1/64). Gates, c state, tmps and the
output are bf16 (2x DVE, half the output DMA); h state is fp8.

Two per-level layouts:
- small levels (<= 256 cols): node order (parent of col j is col j//2), all
  4 k-tiles packed into one psum pair per gate (one activation each), and
  the h-GEMM reads parent h through a stride-0 repeat AP so each parent
  feeds both children directly.
- big levels: block order ([L-children | R-children], parent of col j is
  col j mod npar) so the h-GEMM runs ONCE per parent into the L psum bank;
  a DVE copy evacuates the 64-scaled gh to SBUF and an identity matmul
  accumulates it into the R bank (halves the h-GEMM tensor work). Gates are
  emitted in (i, u, f, o, r) order with the c/tanh chain interleaved so only
  e/hf/h-store trail the last activation, and injects lag their gh copy by
  two gates to hide DVE latency.
"""

import os
import sys

sys.path.insert(0, "/opt/trn_rl_repo")

import numpy as np
import ml_dtypes
from contextlib import ExitStack

import concourse.bass as bass
import concourse.mybir as mybir
import concourse.tile as tile
from concourse import bacc

P = 128
KT = 4               # 512 / 128 contraction tiles
H = 512
F = 512
DEPTH = 17
NCORES = 8
CHUNK = 1024         # children columns per chunk
PCH = 512            # parents per block-order chunk (=> CHUNK children)
M_IOFU = 20          # iofu M-tiles (2560/128), fp8
M_PX = 4             # px M-tiles (512/128), bf16
WSCALE = 64.0        # wx fp8 pre-scale
WHSCALE = 64.0       # wh fp8 pre-scale (h state stored unscaled)
TPACK_W = 256        # pack all 4 k-tiles into one psum pair when w <= this
BF16 = mybir.dt.bfloat16
FP8 = mybir.dt.float8e4
F32 = mybir.dt.float32
AF = mybir.ActivationFunctionType
DR = mybir.MatmulPerfMode.DoubleRow
np_bf16 = ml_dtypes.bfloat16
np_fp8 = ml_dtypes.float8_e4m3


def _level_sizes(depth):
    # per-core column count per level: levels 0..3 replicated, >=4 core-private
    return [1 << d if d <= 3 else 1 << (d - 3) for d in range(depth)]


def _plan(depth):
    """Level sizes, feature-column offsets, and per-parity state-buffer
    widths (level d stores into buffer d % 2; no level splitting — the two
    buffers are sized for the largest even/odd stored level)."""
    Ns = _level_sizes(depth)
    off = [0]
    for n in Ns:
        off.append(off[-1] + n)
    sc = [1, 1]
    for d in range(depth - 1):
        sc[d % 2] = max(sc[d % 2], Ns[d])
    return Ns, off, sc


def build_nc(depth=DEPTH):
    """Build the SPMD single-core Bass program (same NEFF for all 8 cores)."""
    Ns, off, sc = _plan(depth)
    C = off[-1]

    nc = bacc.Bacc("TRN2", target_bir_lowering=False, debug=False)
    featsq = nc.declare_dram_parameter("featsq", [F, C], FP8, isOutput=False)
    featsb = nc.declare_dram_parameter("featsb", [F, C], BF16, isOutput=False)
    wxq = nc.declare_dram_parameter("wxq", [F, M_IOFU * P], FP8, isOutput=False)
    wxpx = nc.declare_dram_parameter("wxpx", [F, M_PX * P], BF16, isOutput=False)
    whq = nc.declare_dram_parameter("whq", [H, M_IOFU * P], FP8, isOutput=False)
    biasm = nc.declare_dram_parameter("biasm", [P, M_IOFU], F32, isOutput=False)
    pxb = nc.declare_dram_parameter("pxb", [P, M_PX], F32, isOutput=False)
    ident = nc.declare_dram_parameter("ident", [P, P], BF16, isOutput=False)
    outT = nc.declare_dram_parameter("outT", [H, C], BF16, isOutput=True)

    featsq_r = featsq[:].rearrange("(a p) c -> p a c", p=P)
    featsb_r = featsb[:].rearrange("(a p) c -> p a c", p=P)
    wxq_r = wxq[:].rearrange("(a p) m -> p a m", p=P)
    wxpx_r = wxpx[:].rearrange("(a p) m -> p a m", p=P)
    whq_r = whq[:].rearrange("(a p) m -> p a m", p=P)
    outT_r = outT[:].rearrange("(a p) c -> p a c", p=P)

    with ExitStack() as ctx:
        tc = ctx.enter_context(tile.TileContext(nc))
        wpool = ctx.enter_context(tc.tile_pool(name="w", bufs=1))
        spool = ctx.enter_context(tc.tile_pool(name="state", bufs=1))
        fpool = ctx.enter_context(tc.tile_pool(name="feats", bufs=3))
        pspool = ctx.enter_context(tc.tile_pool(name="ps", bufs=4, space="PSUM"))
        gpool = ctx.enter_context(tc.tile_pool(name="gates", bufs=10))
        xpool = ctx.enter_context(tc.tile_pool(name="px", bufs=4))
        tpool = ctx.enter_context(tc.tile_pool(name="tmp", bufs=8))
        opool = ctx.enter_context(tc.tile_pool(name="hf", bufs=4))
        ghpool = ctx.enter_context(tc.tile_pool(name="gh", bufs=4))

        wx_sb = wpool.tile([P, KT, M_IOFU * P], FP8, tag="wxq")
        wpx_sb = wpool.tile([P, KT, M_PX * P], BF16, tag="wxpx")
        wh_sb = wpool.tile([P, KT, M_IOFU * P], FP8, tag="whq")
        bias_sb = wpool.tile([P, M_IOFU], F32, tag="biasm")
        pxb_sb = wpool.tile([P, M_PX], F32, tag="pxb")
        id_sb = wpool.tile([P, P], BF16, tag="ident")
        # small tensors first, big weights split by k-tile pair so the first
        # matmuls (kp=0) can start after half the weight traffic
        nc.sync.dma_start(bias_sb[:], biasm[:])
        nc.sync.dma_start(pxb_sb[:], pxb[:])
        nc.sync.dma_start(id_sb[:], ident[:])
        nc.sync.dma_start(wx_sb[:, 0:2, :], wxq_r[:, 0:2, :])
        nc.sync.dma_start(wpx_sb[:, 0:2, :], wxpx_r[:, 0:2, :])
        nc.sync.dma_start(wh_sb[:, 0:2, :], whq_r[:, 0:2, :])
        nc.sync.dma_start(wx_sb[:, 2:4, :], wxq_r[:, 2:4, :])
        nc.sync.dma_start(wpx_sb[:, 2:4, :], wxpx_r[:, 2:4, :])
        nc.sync.dma_start(wh_sb[:, 2:4, :], whq_r[:, 2:4, :])

        # state double buffers (level d -> buffer d % 2): c bf16, h fp8
        cst = [spool.tile([P, KT, sc[b]], BF16, tag=f"c{b}", name=f"c{b}")
               for b in (0, 1)]
        hst = [spool.tile([P, KT, sc[b]], FP8, tag=f"h{b}", name=f"h{b}")
               for b in (0, 1)]

        def elemwise_early(d, w, gi_, gu_, gf_, px, pc_reps, c_dst, tv):
            """First part of the bf16 elementwise chain (after i/u/f acts):
            c = i*u + f*pc and tanh(c). Runs while o/r GEMMs proceed."""
            if d > 0:
                t1 = tpool.tile([P, CHUNK], BF16, tag="tmp")
                t2 = tpool.tile([P, CHUNK], BF16, tag="tmp")
                nc.vector.tensor_mul(t1[:, :w], gi_[:, :w], gu_[:, :w])
                for sl, vfn, pc_rep in pc_reps:
                    nc.gpsimd.tensor_mul(vfn(t2[:, sl]), vfn(gf_[:, sl]),
                                         pc_rep)
                nc.vector.tensor_add(c_dst, tv(t1[:, :w]), tv(t2[:, :w]))
            else:
                nc.vector.tensor_mul(c_dst, tv(gi_[:, :w]), tv(gu_[:, :w]))
            tc_ = tpool.tile([P, CHUNK], BF16, tag="tmp")
            nc.scalar.activation(tv(tc_[:, :w]), c_dst, AF.Tanh)
            return tc_

        def elemwise_mid(w, go_, tc_, px):
            """After o's act: t3 = o*tanh(c); d = t3 - px."""
            t3 = tpool.tile([P, CHUNK], BF16, tag="tmp")
            nc.vector.tensor_mul(t3[:, :w], go_[:, :w], tc_[:, :w])
            d_ = tpool.tile([P, CHUNK], BF16, tag="tmp")
            nc.vector.tensor_sub(d_[:, :w], t3[:, :w], px[:, :w])
            return d_

        def elemwise_late(w, gr_, d_, px, tv, h_dst, out_ap):
            """After r's act (critical tail): e = r*d; hf = e + px; store."""
            e_ = tpool.tile([P, CHUNK], BF16, tag="tmp")
            nc.vector.tensor_mul(e_[:, :w], gr_[:, :w], d_[:, :w])
            hf = opool.tile([P, CHUNK], BF16, tag="hf")
            nc.vector.tensor_add(hf[:, :w], e_[:, :w], px[:, :w])
            if h_dst is not None:
                nc.vector.tensor_copy(h_dst, tv(hf[:, :w]))
            nc.sync.dma_start(out_ap, tv(hf[:, :w]))

        def chunk_tpack(d, col0, st0, pp0, w, store, buf, ftq, ftb):
            """Small-level chunk (w <= TPACK_W): all 4 k-tiles packed into
            one psum pair per gate -> one activation per gate, 3-dim state
            APs, 4x fewer elementwise/activation ops."""
            pbuf = (d - 1) % 2
            hw = w // 2
            tw = KT * w
            tpb = max(1, 512 // w)  # t-slices per 2KB psum bank

            def first_t(t):
                return t % tpb == 0

            def last_t(t):
                return t % tpb == tpb - 1 or t == KT - 1

            ps_px = pspool.tile([P, CHUNK], F32, tag="ps")
            for t in range(KT):
                for k in range(KT):
                    nc.tensor.matmul(
                        ps_px[:, t * w:(t + 1) * w],
                        wpx_sb[:, k, t * P:(t + 1) * P], ftb[:, k, :w],
                        start=(first_t(t) and k == 0),
                        stop=(last_t(t) and k == KT - 1))
            px = xpool.tile([P, CHUNK], BF16, tag="px")
            nc.scalar.activation(px[:, :tw], ps_px[:, :tw], AF.Identity,
                                 bias=pxb_sb[:, 0:1])

            def gate_x(gi, ps):
                """x-parts only (feature-dependent): opens each psum bank."""
                for t in range(KT):
                    m = gi * KT + t
                    for kp in (0, 2):
                        nc.tensor.matmul(
                            ps[:, t * w:(t + 1) * w],
                            wx_sb[:, kp:kp + 2, m * P:(m + 1) * P],
                            ftq[:, kp:kp + 2, :w], perf_mode=DR,
                            start=(first_t(t) and kp == 0),
                            stop=(last_t(t) and kp == 2 and d == 0))

            def gate_h(gi, ps):
                """h-parts (parent-state-dependent): closes the banks."""
                for t in range(KT):
                    m = gi * KT + t
                    for kp in (0, 2):
                        hrep = hst[pbuf][
                            :, kp:kp + 2, pp0:pp0 + hw,
                            None].to_broadcast((P, 2, hw, 2))
                        nc.tensor.matmul(
                            ps[:, t * w:t * w + 2 * hw],
                            wh_sb[:, kp:kp + 2, m * P:(m + 1) * P],
                            hrep, perf_mode=DR,
                            start=False, stop=(last_t(t) and kp == 2))

            def gate_act(gi, ps):
                g = gpool.tile([P, CHUNK], BF16, tag="gates")
                func = AF.Tanh if gi == 3 else AF.Sigmoid
                nc.scalar.activation(g[:, :tw], ps[:, :tw], func,
                                     bias=bias_sb[:, gi * KT:gi * KT + 1],
                                     scale=1.0 / WSCALE)
                return g

            if store:
                c_dst = cst[buf][:, :, st0:st0 + w]
                h_dst = hst[buf][:, :, st0:st0 + w]
            else:
                c_dst = tpool.tile([P, CHUNK], BF16, tag="tmp",
                                   name="ctmp")[:, :tw].rearrange(
                                       "p (a b) -> p a b", a=KT)
                h_dst = None
            tv = lambda ap: ap.rearrange("p (a b) -> p a b", a=KT)
            nodev = lambda ap: ap.rearrange("p (a b) -> p a b", b=2)
            pc_reps = []
            if d > 0:
                for t in range(KT):
                    pc_reps.append((
                        slice(t * w, (t + 1) * w), nodev,
                        cst[pbuf][:, t, pp0:pp0 + hw, None].to_broadcast(
                            (P, hw, 2))))
            # software pipeline: x-parts run ahead (feature-only deps) so the
            # PE has queued work while the parent level's h-store completes
            acted = {}
            pss = {}
            pipe = []

            def retire(g0):
                if d > 0:
                    gate_h(g0, pss[g0])
                acted[g0] = gate_act(g0, pss[g0])

            for gi in [0, 3, 2, 1, 4]:
                if len(pipe) >= 3:
                    retire(pipe.pop(0))
                ps = pspool.tile([P, CHUNK], F32, tag="ps")
                pss[gi] = ps
                gate_x(gi, ps)
                pipe.append(gi)
            retire(pipe.pop(0))  # f
            tc_ = elemwise_early(d, tw, acted[0], acted[3], acted[2], px,
                                 pc_reps, c_dst, tv)
            retire(pipe.pop(0))  # o
            d_ = elemwise_mid(tw, acted[1], tc_, px)
            retire(pipe.pop(0))  # r
            elemwise_late(tw, acted[4], d_, px, tv, h_dst,
                          outT_r[:, :, col0:col0 + w])

        def chunk_blk(d, coloff, npar, p0, pw, store, buf):
            """Block-order chunk for big levels (level size > TPACK_W):
            process parents [p0, p0+pw); children laid out [L-block |
            R-block] within the level. Per gate: h-GEMM once into the L
            half of a 2-bank psum pair, R x-GEMM into the R half as its
            own group, one DVE add seeds gh into R, then the L x-GEMM
            accumulates on top of gh. Halves the h-GEMM tensor work."""
            pbuf = (d - 1) % 2
            w2 = 2 * pw
            ftq = fpool.tile([P, KT, CHUNK], FP8, tag="ftq")
            ftb = fpool.tile([P, KT, CHUNK], BF16, tag="ftb")
            cL = coloff + p0
            cR = coloff + npar + p0
            nc.sync.dma_start(ftq[:, :, 0:pw], featsq_r[:, :, cL:cL + pw])
            nc.sync.dma_start(ftq[:, :, pw:w2], featsq_r[:, :, cR:cR + pw])
            nc.sync.dma_start(ftb[:, :, 0:pw], featsb_r[:, :, cL:cL + pw])
            nc.sync.dma_start(ftb[:, :, pw:w2], featsb_r[:, :, cR:cR + pw])
            for t in range(KT):
                ps_px = pspool.tile([P, CHUNK], F32, tag="ps")
                for k in range(KT):
                    for c0 in range(0, w2, 512):
                        cw = min(512, w2 - c0)
                        nc.tensor.matmul(
                            ps_px[:, c0:c0 + cw],
                            wpx_sb[:, k, t * P:(t + 1) * P],
                            ftb[:, k, c0:c0 + cw],
                            start=(k == 0 and c0 % 512 == 0),
                            stop=(k == KT - 1
                                  and (c0 + cw == w2 or (c0 + cw) % 512 == 0)))
                px = xpool.tile([P, CHUNK], BF16, tag="px")
                nc.scalar.activation(px[:, :w2], ps_px[:, :w2], AF.Identity,
                                     bias=pxb_sb[:, t:t + 1])

                def gate_h_rx(gi, ps):
                    """R-children x-GEMM into [pw:w2] and h-GEMM into [0:pw]
                    (group opened, not closed). When the two halves live in
                    separate psum banks (pw == 512), the Rx part is emitted
                    first: it only needs features, so the in-order PE queue
                    can chew on it while the parent level's h-state store
                    completes."""
                    m = gi * KT + t
                    rx_first = pw % 512 == 0

                    def emit_h():
                        for kp in (0, 2):
                            for q0 in range(0, pw, 256):
                                qw = min(256, pw - q0)
                                nc.tensor.matmul(
                                    ps[:, q0:q0 + qw],
                                    wh_sb[:, kp:kp + 2, m * P:(m + 1) * P],
                                    hst[pbuf][:, kp:kp + 2,
                                              p0 + q0:p0 + q0 + qw],
                                    perf_mode=DR,
                                    start=(kp == 0 and q0 % 512 == 0),
                                    stop=False)

                    def emit_rx():
                        for kp in (0, 2):
                            for c0 in range(pw, w2, 256):
                                cw = min(256, w2 - c0)
                                nc.tensor.matmul(
                                    ps[:, c0:c0 + cw],
                                    wx_sb[:, kp:kp + 2, m * P:(m + 1) * P],
                                    ftq[:, kp:kp + 2, c0:c0 + cw],
                                    perf_mode=DR,
                                    start=(kp == 0 and c0 % 512 == 0),
                                    stop=False)

                    if rx_first:
                        emit_rx()
                        emit_h()
                    else:
                        emit_h()
                        emit_rx()

                def gate_lx(gi, ps):
                    """L-children x-GEMM accumulating onto gh in [0:pw];
                    closes the bank(s) the h-GEMM opened."""
                    m = gi * KT + t
                    for kp in (0, 2):
                        for c0 in range(0, pw, 256):
                            cw = min(256, pw - c0)
                            nc.tensor.matmul(
                                ps[:, c0:c0 + cw],
                                wx_sb[:, kp:kp + 2, m * P:(m + 1) * P],
                                ftq[:, kp:kp + 2, c0:c0 + cw],
                                perf_mode=DR, start=False,
                                stop=(kp == 2 and (c0 + cw == pw
                                                   or (c0 + cw) % 512 == 0)))

                def gate_act(gi, ps):
                    g = gpool.tile([P, CHUNK], BF16, tag="gates")
                    func = AF.Tanh if gi == 3 else AF.Sigmoid
                    nc.scalar.activation(g[:, :w2], ps[:, :w2], func,
                                         bias=bias_sb[:, gi * KT:gi * KT + 1],
                                         scale=1.0 / WSCALE)
                    return g

                if store:
                    c_dst = cst[buf][:, t, 0:2 * npar].rearrange(
                        "p (b q) -> p b q", b=2)[:, :, p0:p0 + pw]
                    h_dst = hst[buf][:, t, 0:2 * npar].rearrange(
                        "p (b q) -> p b q", b=2)[:, :, p0:p0 + pw]
                else:
                    c_dst = tpool.tile([P, CHUNK], BF16, tag="tmp",
                                       name="ctmp")[:, :w2].rearrange(
                                           "p (b q) -> p b q", b=2)
                    h_dst = None
                tv = lambda ap: ap.rearrange("p (b q) -> p b q", b=2)
                out_ap = outT_r[:, t, coloff:coloff + 2 * npar].rearrange(
                    "p (b q) -> p b q", b=2)[:, :, p0:p0 + pw]
                blkv = lambda ap: ap.rearrange("p (b q) -> p b q", b=2)
                pc_reps = [(slice(0, w2), blkv,
                            cst[pbuf][:, t, None, p0:p0 + pw].to_broadcast(
                                (P, 2, pw)))]

                def inject(gi):
                    # accumulate the 64-scaled gh into the R half via an
                    # identity matmul from the SBUF copy
                    nc.tensor.matmul(pss[gi][:, pw:w2], id_sb[:], ghs[gi],
                                     start=False, stop=(pw % 512 == 0))

                # software-pipelined gate sequence (i, u, f, o, r) with
                # 2-gate lag between the gh copy and its inject
                acted = {}
                order = [0, 3, 2, 1, 4]
                pss = {}
                ghs = {}
                pipe = []

                def retire(g0):
                    inject(g0)
                    gate_lx(g0, pss[g0])
                    acted[g0] = gate_act(g0, pss[g0])

                for gi in order:
                    if len(pipe) >= 3:
                        retire(pipe.pop(0))
                    ps = pspool.tile([P, CHUNK], F32, tag="ps")
                    pss[gi] = ps
                    gate_h_rx(gi, ps)
                    gh = ghpool.tile([P, PCH], BF16, tag="gh",
                                     name="gh")[:, :pw]
                    nc.vector.tensor_copy(gh, ps[:, 0:pw])
                    ghs[gi] = gh
                    pipe.append(gi)
                retire(pipe.pop(0))  # f
                tc_ = elemwise_early(d, w2, acted[0], acted[3], acted[2],
                                     px, pc_reps, c_dst, tv)
                retire(pipe.pop(0))  # o
                d_ = elemwise_mid(w2, acted[1], tc_, px)
                retire(pipe.pop(0))  # r
                elemwise_late(w2, acted[4], d_, px, tv, h_dst, out_ap)

        for d in range(depth):
            store = d < depth - 1
            buf = d % 2
            n = Ns[d]
            if n <= TPACK_W:
                ftq = fpool.tile([P, KT, CHUNK], FP8, tag="ftq")
                ftb = fpool.tile([P, KT, CHUNK], BF16, tag="ftb")
                nc.sync.dma_start(ftq[:, :, :n],
                                  featsq_r[:, :, off[d]:off[d] + n])
                nc.sync.dma_start(ftb[:, :, :n],
                                  featsb_r[:, :, off[d]:off[d] + n])
                chunk_tpack(d, off[d], 0, 0, n, store, buf, ftq, ftb)
            else:
                npar = n // 2
                for p0 in range(0, npar, PCH):
                    pw = min(PCH, npar - p0)
                    chunk_blk(d, off[d], npar, p0, pw, store, buf)

    nc.compile()
    return nc, C


# ---------------------------------------------------------------- host side

def _col_maps(depth):
    """Per (core, level): global node index for each column. Small levels
    (size <= TPACK_W) use node order (children interleaved: parent of col j
    is col j//2); big levels use block order ([L-children | R-children]:
    parent of col j is col j mod npar). Levels 0-3 are replicated with a
    per-core child-swap so each core's subtree root lands at column 0 of
    level 3."""
    Ns = _level_sizes(depth)
    maps = []
    for i in range(NCORES):
        per_level = []
        cur = np.array([0], dtype=np.int64)
        for d in range(depth):
            if d == 0:
                cur = np.array([0], dtype=np.int64)
            elif d == 4:
                # first private level: children of this core's subtree root
                r = cur[0]
                cur = np.array([2 * r, 2 * r + 1], dtype=np.int64)
            else:
                L, R = 2 * cur, 2 * cur + 1
                if d <= 3:
                    if (i >> (3 - d)) & 1:
                        L, R = R, L
                    cur = np.stack([L, R], axis=1).ravel()
                elif Ns[d] <= TPACK_W:
                    cur = np.stack([L, R], axis=1).ravel()
                else:
                    cur = np.concatenate([L, R])
            per_level.append(((1 << d) - 1) + cur)
        maps.append(per_level)
    return maps


def prep_inputs(features, px_w, px_b, iofux_w, iofux_b, iofuh_w, iofuh_b,
                depth=DEPTH):
    Ns = _level_sizes(depth)
    C = sum(Ns)
    features = np.asarray(features, np.float32)
    wxq = np.ascontiguousarray(
        (np.asarray(iofux_w, np.float32) * WSCALE).T).astype(np_fp8)
    wxpx = np.ascontiguousarray(
        np.asarray(px_w, np.float32).T).astype(np_bf16)
    whq = np.ascontiguousarray(
        (np.asarray(iofuh_w, np.float32) * WHSCALE).T).astype(np_fp8)
    bias_all = np.asarray(iofux_b, np.float32) + np.asarray(iofuh_b, np.float32)
    biasm = np.ascontiguousarray(bias_all.reshape(M_IOFU, P).T)
    pxbm = np.ascontiguousarray(
        np.asarray(px_b, np.float32).reshape(M_PX, P).T)

    maps = _col_maps(depth)
    idm = np.eye(P, dtype=np_bf16)
    in_maps = []
    for i in range(NCORES):
        cols = np.concatenate(maps[i])
        fcore = features[cols, :]                       # [C, 512] f32
        fT = np.ascontiguousarray(fcore.T)              # [512, C]
        in_maps.append({"featsq": fT.astype(np_fp8),
                        "featsb": fT.astype(np_bf16),
                        "wxq": wxq, "wxpx": wxpx, "whq": whq,
                        "biasm": biasm, "pxb": pxbm, "ident": idm})
    return in_maps, maps, C


def assemble_output(results, maps, depth=DEPTH):
    Ns = _level_sizes(depth)
    n_nodes = (1 << depth) - 1
    out = np.empty((n_nodes, H), np.float32)
    offs = np.cumsum([0] + Ns)
    for i in range(NCORES):
        o = results[i]["outT"]                          # [512, C] bf16
        for d in range(depth):
            if d <= 3 and i != 0:
                continue  # replicated levels: take core 0's copy
            cols = maps[i][d]
            out[cols, :] = o[:, offs[d]:offs[d + 1]].T.astype(np.float32)
    return out


_CACHE = {}


def _get_built(depth=DEPTH):
    if depth not in _CACHE:
        _CACHE[depth] = build_nc(depth)
    return _CACHE[depth]


def run_cores(in_maps, depth=DEPTH, trace=False):
    from concourse.bass_utils import run_bass_kernel_spmd
    nc, C = _get_built(depth)
    br = run_bass_kernel_spmd(nc, in_maps, list(range(NCORES)), trace=trace)
    return br


def kernel(features, px_w, px_b, iofux_w, iofux_b, iofuh_w, iofuh_b):
    in_maps, maps, C = prep_inputs(features, px_w, px_b, iofux_w, iofux_b,
                                   iofuh_w, iofuh_b)
    br = run_cores(in_maps)
    return assemble_output(br.results, maps)


# revision 44
# speedup vs baseline: 1.1998x; 1.0033x over previous
"""Root-to-leaves TreeLSTM over a complete binary tree (depth 17, 131071 nodes,
feat=h=512), distributed over 8 TRN2 NeuronCores with zero inter-core
communication.

Sharding: level d's nodes split into 8 contiguous chunks means each core's
chunk at level d+1 is exactly the children of its chunk at level d, so each
core owns one of the 8 subtrees rooted at level 3. Levels 0-3 are replicated
on all cores; the SPMD program relabels them per-core by XOR with the core
index prefix so "my subtree root" is column 0 everywhere and the parent map
is position-independent (parent of col j is col j//2 in plain node order).

v2: fp8 DoubleRow matmuls (2x PE throughput) for the iofu x-GEMM and h-GEMM,
accumulated into a 2-bank PSUM pair per (gate, k-tile-of-h). The px GEMM
stays bf16 for accuracy (its error passes to the output unattenuated by any
sigmoid). Weights are pre-scaled into fp8's normal range (wx*64, wh*64) and
compensated by the activation scale (1/64). Gates, c state, tmps and the
output are bf16 (2x DVE, half the output DMA); h state is fp8.

Two per-level layouts:
- small levels (<= 256 cols): node order (parent of col j is col j//2), all
  4 k-tiles packed into one psum pair per gate (one activation each), and
  the h-GEMM reads parent h through a stride-0 repeat AP so each parent
  feeds both children directly.
- big levels: block order ([L-children | R-children], parent of col j is
  col j mod npar) so the h-GEMM runs ONCE per parent into the L psum bank;
  a DVE copy evacuates the 64-scaled gh to SBUF and an identity matmul
  accumulates it into the R bank (halves the h-GEMM tensor work). Gates are
  emitted in (i, u, f, o, r) order with the c/tanh chain interleaved so only
  e/hf/h-store trail the last activation, and injects lag their gh copy by
  two gates to hide DVE latency.
"""

import os
import sys

sys.path.insert(0, "/opt/trn_rl_repo")

import numpy as np
import ml_dtypes
from contextlib import ExitStack

import concourse.bass as bass
import concourse.mybir as mybir
import concourse.tile as tile
from concourse import bacc

P = 128
KT = 4               # 512 / 128 contraction tiles
H = 512
F = 512
DEPTH = 17
NCORES = 8
CHUNK = 1024         # children columns per chunk
PCH = 512            # parents per block-order chunk (=> CHUNK children)
M_IOFU = 20          # iofu M-tiles (2560/128), fp8
M_PX = 4             # px M-tiles (512/128), bf16
WSCALE = 64.0        # wx fp8 pre-scale
WHSCALE = 64.0       # wh fp8 pre-scale (h state stored unscaled)
TPACK_W = 256        # pack all 4 k-tiles into one psum pair when w <= this
BF16 = mybir.dt.bfloat16
FP8 = mybir.dt.float8e4
F32 = mybir.dt.float32
AF = mybir.ActivationFunctionType
DR = mybir.MatmulPerfMode.DoubleRow
np_bf16 = ml_dtypes.bfloat16
np_fp8 = ml_dtypes.float8_e4m3


def _level_sizes(depth):
    # per-core column count per level: levels 0..3 replicated, >=4 core-private
    return [1 << d if d <= 3 else 1 << (d - 3) for d in range(depth)]


def _plan(depth):
    """Level sizes, feature-column offsets, and per-parity state-buffer
    widths (level d stores into buffer d % 2; no level splitting — the two
    buffers are sized for the largest even/odd stored level)."""
    Ns = _level_sizes(depth)
    off = [0]
    for n in Ns:
        off.append(off[-1] + n)
    sc = [1, 1]
    for d in range(depth - 1):
        sc[d % 2] = max(sc[d % 2], Ns[d])
    return Ns, off, sc


def build_nc(depth=DEPTH):
    """Build the SPMD single-core Bass program (same NEFF for all 8 cores)."""
    Ns, off, sc = _plan(depth)
    C = off[-1]

    nc = bacc.Bacc("TRN2", target_bir_lowering=False, debug=False)
    featsq = nc.declare_dram_parameter("featsq", [F, C], FP8, isOutput=False)
    featsb = nc.declare_dram_parameter("featsb", [F, C], BF16, isOutput=False)
    wxq = nc.declare_dram_parameter("wxq", [F, M_IOFU * P], FP8, isOutput=False)
    wxpx = nc.declare_dram_parameter("wxpx", [F, M_PX * P], BF16, isOutput=False)
    whq = nc.declare_dram_parameter("whq", [H, M_IOFU * P], FP8, isOutput=False)
    biasm = nc.declare_dram_parameter("biasm", [P, M_IOFU], F32, isOutput=False)
    pxb = nc.declare_dram_parameter("pxb", [P, M_PX], F32, isOutput=False)
    ident = nc.declare_dram_parameter("ident", [P, P], BF16, isOutput=False)
    outT = nc.declare_dram_parameter("outT", [H, C], BF16, isOutput=True)

    featsq_r = featsq[:].rearrange("(a p) c -> p a c", p=P)
    featsb_r = featsb[:].rearrange("(a p) c -> p a c", p=P)
    wxq_r = wxq[:].rearrange("(a p) m -> p a m", p=P)
    wxpx_r = wxpx[:].rearrange("(a p) m -> p a m", p=P)
    whq_r = whq[:].rearrange("(a p) m -> p a m", p=P)
    outT_r = outT[:].rearrange("(a p) c -> p a c", p=P)

    with ExitStack() as ctx:
        tc = ctx.enter_context(tile.TileContext(nc))
        wpool = ctx.enter_context(tc.tile_pool(name="w", bufs=1))
        spool = ctx.enter_context(tc.tile_pool(name="state", bufs=1))
        fpool = ctx.enter_context(tc.tile_pool(name="feats", bufs=3))
        pspool = ctx.enter_context(tc.tile_pool(name="ps", bufs=4, space="PSUM"))
        gpool = ctx.enter_context(tc.tile_pool(name="gates", bufs=10))
        xpool = ctx.enter_context(tc.tile_pool(name="px", bufs=4))
        tpool = ctx.enter_context(tc.tile_pool(name="tmp", bufs=8))
        opool = ctx.enter_context(tc.tile_pool(name="hf", bufs=4))
        ghpool = ctx.enter_context(tc.tile_pool(name="gh", bufs=4))

        wx_sb = wpool.tile([P, KT, M_IOFU * P], FP8, tag="wxq")
        wpx_sb = wpool.tile([P, KT, M_PX * P], BF16, tag="wxpx")
        wh_sb = wpool.tile([P, KT, M_IOFU * P], FP8, tag="whq")
        bias_sb = wpool.tile([P, M_IOFU], F32, tag="biasm")
        pxb_sb = wpool.tile([P, M_PX], F32, tag="pxb")
        id_sb = wpool.tile([P, P], BF16, tag="ident")
        # small tensors first, big weights split by k-tile pair so the first
        # matmuls (kp=0) can start after half the weight traffic
        nc.sync.dma_start(bias_sb[:], biasm[:])
        nc.sync.dma_start(pxb_sb[:], pxb[:])
        nc.sync.dma_start(id_sb[:], ident[:])
        nc.sync.dma_start(wx_sb[:, 0:2, :], wxq_r[:, 0:2, :])
        nc.sync.dma_start(wpx_sb[:, 0:2, :], wxpx_r[:, 0:2, :])
        nc.sync.dma_start(wh_sb[:, 0:2, :], whq_r[:, 0:2, :])
        nc.sync.dma_start(wx_sb[:, 2:4, :], wxq_r[:, 2:4, :])
        nc.sync.dma_start(wpx_sb[:, 2:4, :], wxpx_r[:, 2:4, :])
        nc.sync.dma_start(wh_sb[:, 2:4, :], whq_r[:, 2:4, :])

        # state double buffers (level d -> buffer d % 2): c bf16, h fp8
        cst = [spool.tile([P, KT, sc[b]], BF16, tag=f"c{b}", name=f"c{b}")
               for b in (0, 1)]
        hst = [spool.tile([P, KT, sc[b]], FP8, tag=f"h{b}", name=f"h{b}")
               for b in (0, 1)]

        def elemwise_early(d, w, gi_, gu_, gf_, px, pc_reps, c_dst, tv):
            """First part of the bf16 elementwise chain (after i/u/f acts):
            c = i*u + f*pc and tanh(c). Runs while o/r GEMMs proceed."""
            if d > 0:
                t1 = tpool.tile([P, CHUNK], BF16, tag="tmp")
                t2 = tpool.tile([P, CHUNK], BF16, tag="tmp")
                nc.vector.tensor_mul(t1[:, :w], gi_[:, :w], gu_[:, :w])
                for sl, vfn, pc_rep in pc_reps:
                    nc.gpsimd.tensor_mul(vfn(t2[:, sl]), vfn(gf_[:, sl]),
                                         pc_rep)
                nc.vector.tensor_add(c_dst, tv(t1[:, :w]), tv(t2[:, :w]))
            else:
                nc.vector.tensor_mul(c_dst, tv(gi_[:, :w]), tv(gu_[:, :w]))
            tc_ = tpool.tile([P, CHUNK], BF16, tag="tmp")
            nc.scalar.activation(tv(tc_[:, :w]), c_dst, AF.Tanh)
            return tc_

        def elemwise_mid(w, go_, tc_, px):
            """After o's act: t3 = o*tanh(c); d = t3 - px."""
            t3 = tpool.tile([P, CHUNK], BF16, tag="tmp")
            nc.vector.tensor_mul(t3[:, :w], go_[:, :w], tc_[:, :w])
            d_ = tpool.tile([P, CHUNK], BF16, tag="tmp")
            nc.vector.tensor_sub(d_[:, :w], t3[:, :w], px[:, :w])
            return d_

        def elemwise_late(w, gr_, d_, px, tv, h_dst, out_ap):
            """After r's act (critical tail): e = r*d; hf = e + px; store."""
            e_ = tpool.tile([P, CHUNK], BF16, tag="tmp")
            nc.vector.tensor_mul(e_[:, :w], gr_[:, :w], d_[:, :w])
            hf = opool.tile([P, CHUNK], BF16, tag="hf")
            nc.vector.tensor_add(hf[:, :w], e_[:, :w], px[:, :w])
            if h_dst is not None:
                nc.vector.tensor_copy(h_dst, tv(hf[:, :w]))
            nc.sync.dma_start(out_ap, tv(hf[:, :w]))

        def chunk_tpack(d, col0, st0, pp0, w, store, buf, ftq, ftb):
            """Small-level chunk (w <= TPACK_W): all 4 k-tiles packed into
            one psum pair per gate -> one activation per gate, 3-dim state
            APs, 4x fewer elementwise/activation ops."""
            pbuf = (d - 1) % 2
            hw = w // 2
            tw = KT * w
            tpb = max(1, 512 // w)  # t-slices per 2KB psum bank

            def first_t(t):
                return t % tpb == 0

            def last_t(t):
                return t % tpb == tpb - 1 or t == KT - 1

            ps_px = pspool.tile([P, CHUNK], F32, tag="ps")
            for t in range(KT):
                for k in range(KT):
                    nc.tensor.matmul(
                        ps_px[:, t * w:(t + 1) * w],
                        wpx_sb[:, k, t * P:(t + 1) * P], ftb[:, k, :w],
                        start=(first_t(t) and k == 0),
                        stop=(last_t(t) and k == KT - 1))
            px = xpool.tile([P, CHUNK], BF16, tag="px")
            nc.scalar.activation(px[:, :tw], ps_px[:, :tw], AF.Identity,
                                 bias=pxb_sb[:, 0:1])

            def gate_x(gi, ps):
                """x-parts only (feature-dependent): opens each psum bank."""
                for t in range(KT):
                    m = gi * KT + t
                    for kp in (0, 2):
                        nc.tensor.matmul(
                            ps[:, t * w:(t + 1) * w],
                            wx_sb[:, kp:kp + 2, m * P:(m + 1) * P],
                            ftq[:, kp:kp + 2, :w], perf_mode=DR,
                            start=(first_t(t) and kp == 0),
                            stop=(last_t(t) and kp == 2 and d == 0))

            def gate_h(gi, ps):
                """h-parts (parent-state-dependent): closes the banks."""
                for t in range(KT):
                    m = gi * KT + t
                    for kp in (0, 2):
                        hrep = hst[pbuf][
                            :, kp:kp + 2, pp0:pp0 + hw,
                            None].to_broadcast((P, 2, hw, 2))
                        nc.tensor.matmul(
                            ps[:, t * w:t * w + 2 * hw],
                            wh_sb[:, kp:kp + 2, m * P:(m + 1) * P],
                            hrep, perf_mode=DR,
                            start=False, stop=(last_t(t) and kp == 2))

            def gate_act(gi, ps):
                g = gpool.tile([P, CHUNK], BF16, tag="gates")
                func = AF.Tanh if gi == 3 else AF.Sigmoid
                nc.scalar.activation(g[:, :tw], ps[:, :tw], func,
                                     bias=bias_sb[:, gi * KT:gi * KT + 1],
                                     scale=1.0 / WSCALE)
                return g

            if store:
                c_dst = cst[buf][:, :, st0:st0 + w]
                h_dst = hst[buf][:, :, st0:st0 + w]
            else:
                c_dst = tpool.tile([P, CHUNK], BF16, tag="tmp",
                                   name="ctmp")[:, :tw].rearrange(
                                       "p (a b) -> p a b", a=KT)
                h_dst = None
            tv = lambda ap: ap.rearrange("p (a b) -> p a b", a=KT)
            nodev = lambda ap: ap.rearrange("p (a b) -> p a b", b=2)
            pc_reps = []
            if d > 0:
                for t in range(KT):
                    pc_reps.append((
                        slice(t * w, (t + 1) * w), nodev,
                        cst[pbuf][:, t, pp0:pp0 + hw, None].to_broadcast(
                            (P, hw, 2))))
            # software pipeline: x-parts run ahead (feature-only deps) so the
            # PE has queued work while the parent level's h-store completes
            acted = {}
            pss = {}
            pipe = []

            def retire(g0):
                if d > 0:
                    gate_h(g0, pss[g0])
                acted[g0] = gate_act(g0, pss[g0])

            for gi in [0, 3, 2, 1, 4]:
                if len(pipe) >= 3:
                    retire(pipe.pop(0))
                ps = pspool.tile([P, CHUNK], F32, tag="ps")
                pss[gi] = ps
                gate_x(gi, ps)
                pipe.append(gi)
            retire(pipe.pop(0))  # f
            tc_ = elemwise_early(d, tw, acted[0], acted[3], acted[2], px,
                                 pc_reps, c_dst, tv)
            retire(pipe.pop(0))  # o
            d_ = elemwise_mid(tw, acted[1], tc_, px)
            retire(pipe.pop(0))  # r
            elemwise_late(tw, acted[4], d_, px, tv, h_dst,
                          outT_r[:, :, col0:col0 + w])

        def chunk_blk(d, coloff, npar, p0, pw, store, buf):
            """Block-order chunk for big levels (level size > TPACK_W):
            process parents [p0, p0+pw); children laid out [L-block |
            R-block] within the level. Per gate: h-GEMM once into the L
            half of a 2-bank psum pair, R x-GEMM into the R half as its
            own group, one DVE add seeds gh into R, then the L x-GEMM
            accumulates on top of gh. Halves the h-GEMM tensor work."""
            pbuf = (d - 1) % 2
            w2 = 2 * pw
            ftq = fpool.tile([P, KT, CHUNK], FP8, tag="ftq")
            ftb = fpool.tile([P, KT, CHUNK], BF16, tag="ftb")
            cL = coloff + p0
            cR = coloff + npar + p0
            nc.sync.dma_start(ftq[:, :, 0:pw], featsq_r[:, :, cL:cL + pw])
            nc.sync.dma_start(ftq[:, :, pw:w2], featsq_r[:, :, cR:cR + pw])
            nc.sync.dma_start(ftb[:, :, 0:pw], featsb_r[:, :, cL:cL + pw])
            nc.sync.dma_start(ftb[:, :, pw:w2], featsb_r[:, :, cR:cR + pw])
            for t in range(KT):
                ps_px = pspool.tile([P, CHUNK], F32, tag="ps")
                for k in range(KT):
                    for c0 in range(0, w2, 512):
                        cw = min(512, w2 - c0)
                        nc.tensor.matmul(
                            ps_px[:, c0:c0 + cw],
                            wpx_sb[:, k, t * P:(t + 1) * P],
                            ftb[:, k, c0:c0 + cw],
                            start=(k == 0 and c0 % 512 == 0),
                            stop=(k == KT - 1
                                  and (c0 + cw == w2 or (c0 + cw) % 512 == 0)))
                px = xpool.tile([P, CHUNK], BF16, tag="px")
                nc.scalar.activation(px[:, :w2], ps_px[:, :w2], AF.Identity,
                                     bias=pxb_sb[:, t:t + 1])

                def gate_rx(gi, ps):
                    """R-children x-GEMM into [pw:w2] (feature-only deps;
                    opens its bank — including the shared bank when
                    pw < 512, since Rx now precedes the h-GEMM)."""
                    m = gi * KT + t
                    for kp in (0, 2):
                        for c0 in range(pw, w2, 256):
                            cw = min(256, w2 - c0)
                            nc.tensor.matmul(
                                ps[:, c0:c0 + cw],
                                wx_sb[:, kp:kp + 2, m * P:(m + 1) * P],
                                ftq[:, kp:kp + 2, c0:c0 + cw],
                                perf_mode=DR,
                                start=(kp == 0
                                       and (c0 % 512 == 0 or c0 == pw)),
                                stop=False)

                def gate_h(gi, ps):
                    """h-GEMM into [0:pw]; opens bank A only when it is not
                    shared with the (already started) R half."""
                    m = gi * KT + t
                    for kp in (0, 2):
                        for q0 in range(0, pw, 256):
                            qw = min(256, pw - q0)
                            nc.tensor.matmul(
                                ps[:, q0:q0 + qw],
                                wh_sb[:, kp:kp + 2, m * P:(m + 1) * P],
                                hst[pbuf][:, kp:kp + 2,
                                          p0 + q0:p0 + q0 + qw],
                                perf_mode=DR,
                                start=(kp == 0 and q0 == 0
                                       and pw % 512 == 0),
                                stop=False)

                def gate_lx(gi, ps):
                    """L-children x-GEMM accumulating onto gh in [0:pw];
                    closes the bank(s) the h-GEMM opened."""
                    m = gi * KT + t
                    for kp in (0, 2):
                        for c0 in range(0, pw, 256):
                            cw = min(256, pw - c0)
                            nc.tensor.matmul(
                                ps[:, c0:c0 + cw],
                                wx_sb[:, kp:kp + 2, m * P:(m + 1) * P],
                                ftq[:, kp:kp + 2, c0:c0 + cw],
                                perf_mode=DR, start=False,
                                stop=(kp == 2 and (c0 + cw == pw
                                                   or (c0 + cw) % 512 == 0)))

                def gate_act(gi, ps):
                    g = gpool.tile([P, CHUNK], BF16, tag="gates")
                    func = AF.Tanh if gi == 3 else AF.Sigmoid
                    nc.scalar.activation(g[:, :w2], ps[:, :w2], func,
                                         bias=bias_sb[:, gi * KT:gi * KT + 1],
                                         scale=1.0 / WSCALE)
                    return g

                if store:
                    c_dst = cst[buf][:, t, 0:2 * npar].rearrange(
                        "p (b q) -> p b q", b=2)[:, :, p0:p0 + pw]
                    h_dst = hst[buf][:, t, 0:2 * npar].rearrange(
                        "p (b q) -> p b q", b=2)[:, :, p0:p0 + pw]
                else:
                    c_dst = tpool.tile([P, CHUNK], BF16, tag="tmp",
                                       name="ctmp")[:, :w2].rearrange(
                                           "p (b q) -> p b q", b=2)
                    h_dst = None
                tv = lambda ap: ap.rearrange("p (b q) -> p b q", b=2)
                out_ap = outT_r[:, t, coloff:coloff + 2 * npar].rearrange(
                    "p (b q) -> p b q", b=2)[:, :, p0:p0 + pw]
                blkv = lambda ap: ap.rearrange("p (b q) -> p b q", b=2)
                pc_reps = [(slice(0, w2), blkv,
                            cst[pbuf][:, t, None, p0:p0 + pw].to_broadcast(
                                (P, 2, pw)))]

                def inject(gi):
                    # accumulate the 64-scaled gh into the R half via an
                    # identity matmul from the SBUF copy
                    nc.tensor.matmul(pss[gi][:, pw:w2], id_sb[:], ghs[gi],
                                     start=False, stop=(pw % 512 == 0))

                # software-pipelined gate sequence (i, u, f, o, r):
                # Rx at lag 0 (feature-only), h + gh-copy at lag 1,
                # inject + Lx + act at lag 2 — maximizes the PE's runnable
                # runahead while the parent level's h-state store completes
                acted = {}
                seq = [0, 3, 2, 1, 4]
                pss = {}
                ghs = {}

                def stage1(g0):
                    gate_h(g0, pss[g0])
                    gh = ghpool.tile([P, PCH], BF16, tag="gh",
                                     name="gh")[:, :pw]
                    nc.vector.tensor_copy(gh, pss[g0][:, 0:pw])
                    ghs[g0] = gh

                def stage2(g0):
                    inject(g0)
                    gate_lx(g0, pss[g0])
                    acted[g0] = gate_act(g0, pss[g0])

                for idx, gi in enumerate(seq):
                    ps = pspool.tile([P, CHUNK], F32, tag="ps")
                    pss[gi] = ps
                    gate_rx(gi, ps)
                    if idx >= 1:
                        stage1(seq[idx - 1])
                    if idx >= 2:
                        stage2(seq[idx - 2])
                stage1(seq[4])   # r's h + copy
                stage2(seq[3])   # o
                tc_ = elemwise_early(d, w2, acted[0], acted[3], acted[2],
                                     px, pc_reps, c_dst, tv)
                stage2(seq[4])   # r
                d_ = elemwise_mid(w2, acted[1], tc_, px)
                elemwise_late(w2, acted[4], d_, px, tv, h_dst, out_ap)

        for d in range(depth):
            store = d < depth - 1
            buf = d % 2
            n = Ns[d]
            if n <= TPACK_W:
                ftq = fpool.tile([P, KT, CHUNK], FP8, tag="ftq")
                ftb = fpool.tile([P, KT, CHUNK], BF16, tag="ftb")
                nc.sync.dma_start(ftq[:, :, :n],
                                  featsq_r[:, :, off[d]:off[d] + n])
                nc.sync.dma_start(ftb[:, :, :n],
                                  featsb_r[:, :, off[d]:off[d] + n])
                chunk_tpack(d, off[d], 0, 0, n, store, buf, ftq, ftb)
            else:
                npar = n // 2
                for p0 in range(0, npar, PCH):
                    pw = min(PCH, npar - p0)
                    chunk_blk(d, off[d], npar, p0, pw, store, buf)

    nc.compile()
    return nc, C


# ---------------------------------------------------------------- host side

def _col_maps(depth):
    """Per (core, level): global node index for each column. Small levels
    (size <= TPACK_W) use node order (children interleaved: parent of col j
    is col j//2); big levels use block order ([L-children | R-children]:
    parent of col j is col j mod npar). Levels 0-3 are replicated with a
    per-core child-swap so each core's subtree root lands at column 0 of
    level 3."""
    Ns = _level_sizes(depth)
    maps = []
    for i in range(NCORES):
        per_level = []
        cur = np.array([0], dtype=np.int64)
        for d in range(depth):
            if d == 0:
                cur = np.array([0], dtype=np.int64)
            elif d == 4:
                # first private level: children of this core's subtree root
                r = cur[0]
                cur = np.array([2 * r, 2 * r + 1], dtype=np.int64)
            else:
                L, R = 2 * cur, 2 * cur + 1
                if d <= 3:
                    if (i >> (3 - d)) & 1:
                        L, R = R, L
                    cur = np.stack([L, R], axis=1).ravel()
                elif Ns[d] <= TPACK_W:
                    cur = np.stack([L, R], axis=1).ravel()
                else:
                    cur = np.concatenate([L, R])
            per_level.append(((1 << d) - 1) + cur)
        maps.append(per_level)
    return maps


def prep_inputs(features, px_w, px_b, iofux_w, iofux_b, iofuh_w, iofuh_b,
                depth=DEPTH):
    Ns = _level_sizes(depth)
    C = sum(Ns)
    features = np.asarray(features, np.float32)
    wxq = np.ascontiguousarray(
        (np.asarray(iofux_w, np.float32) * WSCALE).T).astype(np_fp8)
    wxpx = np.ascontiguousarray(
        np.asarray(px_w, np.float32).T).astype(np_bf16)
    whq = np.ascontiguousarray(
        (np.asarray(iofuh_w, np.float32) * WHSCALE).T).astype(np_fp8)
    bias_all = np.asarray(iofux_b, np.float32) + np.asarray(iofuh_b, np.float32)
    biasm = np.ascontiguousarray(bias_all.reshape(M_IOFU, P).T)
    pxbm = np.ascontiguousarray(
        np.asarray(px_b, np.float32).reshape(M_PX, P).T)

    maps = _col_maps(depth)
    idm = np.eye(P, dtype=np_bf16)
    in_maps = []
    for i in range(NCORES):
        cols = np.concatenate(maps[i])
        fcore = features[cols, :]                       # [C, 512] f32
        fT = np.ascontiguousarray(fcore.T)              # [512, C]
        in_maps.append({"featsq": fT.astype(np_fp8),
                        "featsb": fT.astype(np_bf16),
                        "wxq": wxq, "wxpx": wxpx, "whq": whq,
                        "biasm": biasm, "pxb": pxbm, "ident": idm})
    return in_maps, maps, C


def assemble_output(results, maps, depth=DEPTH):
    Ns = _level_sizes(depth)
    n_nodes = (1 << depth) - 1
    out = np.empty((n_nodes, H), np.float32)
    offs = np.cumsum([0] + Ns)
    for i in range(NCORES):
        o = results[i]["outT"]                          # [512, C] bf16
        for d in range(depth):
            if d <= 3 and i != 0:
                continue  # replicated levels: take core 0's copy
            cols = maps[i][d]
            out[cols, :] = o[:, offs[d]:offs[d + 1]].T.astype(np.float32)
    return out


_CACHE = {}


def _get_built(depth=DEPTH):
    if depth not in _CACHE:
        _CACHE[depth] = build_nc(depth)
    return _CACHE[depth]


def run_cores(in_maps, depth=DEPTH, trace=False):
    from concourse.bass_utils import run_bass_kernel_spmd
    nc, C = _get_built(depth)
    br = run_bass_kernel_spmd(nc, in_maps, list(range(NCORES)), trace=trace)
    return br


def kernel(features, px_w, px_b, iofux_w, iofux_b, iofuh_w, iofuh_b):
    in_maps, maps, C = prep_inputs(features, px_w, px_b, iofux_w, iofux_b,
                                   iofuh_w, iofuh_b)
    br = run_cores(in_maps)
    return assemble_output(br.results, maps)
